# revision 11
# baseline (speedup 1.0000x reference)
"""Trainium2 Bass kernel for EvenNet GNN message passing, SPMD across 8 NeuronCores.

v2 strategy (vs baseline): the baseline was bottlenecked by SWDGE descriptor
generation on the GpSimd/Pool engine (measured 4.7us/call + 1.5ns/idx,
98 calls/hop, 253k idx/hop -> 8.5ms of 12.3ms).  This version:

  - bf16 pair-row feature table: row p = [h(2p) | h(2p+1)] (128 bf16 =
    256B).  Pair count 25600 < 32768 so a single int16 index window covers
    the table (the baseline needed a two-window A/B split with balanced
    padding: 253k -> ~210k idx).
  - Per-slot bf16 weights w = dinv[src] * (parity mask of the wanted
    half); the unwanted half and all pad slots get weight 0.  The slot
    reduce becomes: multiply the gathered [128, C, 128] tile (viewed as
    [128, NT, 2L, 64]) by broadcast weights, then an in-place bf16
    halving-tree over the slot axis.  Few large DVE instructions instead
    of many small ones.
  - Gather calls batched over many tiles (~8 calls/hop instead of 98).
  - bf16 MLP (4x tensor throughput), bf16 AllGather (half the bytes).
  - log_softmax with broadcast ops (6 instrs instead of ~100).

Layout: nodes degree-sorted per core; position pos in [0, 6400);
partition p = pos % 128, tile g = pos // 128 (G = 50 tiles).  Table pair
row (global) = core*3200 + p*25 + (g>>1); half = g & 1.  hn SBUF tile
[128, 50, 64] bf16 dumps per-partition-contiguously = exactly the pair
table shard layout.
"""

import numpy as np

try:
    import ml_dtypes
    BF16 = ml_dtypes.bfloat16
except ImportError:  # pragma: no cover
    BF16 = None

N_CORES = 8
P = 128
CLS = 64
COLS_PER_BATCH = 224  # max NT*L slot columns per gather call (SBUF budget)


# ---------------------------------------------------------------------------
# Host preprocessing
# ---------------------------------------------------------------------------

def preprocess(x, edge_index, W1, b1, W2, b2, gamma, n_cores=N_CORES):
    x = np.ascontiguousarray(np.asarray(x, np.float32))
    edge_index = np.asarray(edge_index)
    W1 = np.asarray(W1, np.float32)
    b1 = np.asarray(b1, np.float32)
    W2 = np.asarray(W2, np.float32)
    b2 = np.asarray(b2, np.float32)
    gamma = np.asarray(gamma, np.float32)

    N, F_IN = x.shape
    HID = W1.shape[1]
    K = gamma.shape[0] - 1
    assert N % n_cores == 0
    NPC = N // n_cores
    G = -(-(NPC + 1) // (2 * P)) * 2  # tiles, rounded to even (pair rows)
    NPC_PAD = G * P
    PAIRS_PC = NPC_PAD // 2          # pair rows per core
    V2 = n_cores * PAIRS_PC          # total pair rows
    assert V2 < 32768, V2

    src = edge_index[0].astype(np.int64)
    dst = edge_index[1].astype(np.int64)
    E = src.shape[0]

    deg = (np.bincount(dst, minlength=N) + 1.0).astype(np.float32)  # + self
    dinv = (1.0 / np.sqrt(deg)).astype(np.float32)

    # degree-sorted positions per core
    perms = []
    pos_of_node = np.empty(N, np.int64)
    for c in range(n_cores):
        lo = c * NPC
        order = np.argsort(-deg[lo:lo + NPC], kind="stable")
        perm = np.full(NPC_PAD, -1, np.int64)
        perm[:NPC] = order
        perms.append(perm)
        invp = np.empty(NPC, np.int64)
        invp[order] = np.arange(NPC)
        pos_of_node[lo:lo + NPC] = invp

    core_of_node = np.arange(N) // NPC
    p_of_node = pos_of_node % P
    g_of_node = pos_of_node // P
    # global pair row + half for every node (as gather source)
    pairrow_of_node = (core_of_node * PAIRS_PC + p_of_node * (G // 2)
                       + (g_of_node >> 1)).astype(np.int32)
    half_of_node = (g_of_node & 1).astype(np.int64)

    # per-edge dst coordinates
    ecore = core_of_node[dst]
    dp = p_of_node[dst]
    dg = g_of_node[dst]

    # slot index within dst, ordered by source pair row (HBM locality)
    okey = (dst << 16) | pairrow_of_node[src].astype(np.int64)
    order_e = np.argsort(okey, kind="stable")
    sd = dst[order_e]
    first = np.r_[True, sd[1:] != sd[:-1]]
    idx_first = np.where(first)[0]
    run_starts = np.repeat(idx_first, np.diff(np.r_[idx_first, len(sd)]))
    l_sorted = np.arange(len(sd)) - run_starts
    l_arr = np.empty(E, np.int64)
    l_arr[order_e] = l_sorted

    # per-tile slot count L[g]: max over cores and partitions
    degi = deg.astype(np.int64) - 1  # in-edge count (no self)
    Lg = np.zeros(G, np.int64)
    np.maximum.at(Lg, g_of_node, degi)
    Lg = np.maximum(Lg, 1)

    # greedy batches of consecutive tiles (degree-desc => L non-increasing)
    batches = []  # (g0, NT, Lb)
    g0 = 0
    while g0 < G:
        Lb = Lg[g0]
        nt = 1
        while g0 + nt < G and (nt + 1) * Lb <= COLS_PER_BATCH:
            nt += 1
        batches.append((g0, nt, int(Lb)))
        g0 += nt

    # column offsets
    col_off = np.zeros(G, np.int64)      # slot-column base of tile g
    bat_base = []                        # idx-column base of each batch
    acc = 0
    for (g0, nt, lb) in batches:
        bat_base.append(acc)
        for i in range(nt):
            col_off[g0 + i] = acc + i * lb
        acc += nt * lb
    SUMC = acc                           # total slot columns

    # gather calls are chunks of <= CHUNK_COLS columns (SWDGE ring capacity)
    CHUNK_COLS = 61
    chunks = []   # (batch_idx, c0, c1, wrapped_col_base)
    wacc = 0
    for bi, (g0, nt, lb) in enumerate(batches):
        cb = nt * lb
        c0 = 0
        while c0 < cb:
            c1 = min(c0 + CHUNK_COLS, cb)
            chunks.append((bi, c0, c1, wacc))
            wacc += 8 * (c1 - c0)
            c0 = c1
    WSUM = wacc

    # idx and weight images
    idx_img = np.zeros((n_cores, P, SUMC), np.int32)
    w_img = np.zeros((n_cores, P, 2 * SUMC), np.float32)
    ecol = col_off[dg] + l_arr           # slot column of each edge
    idx_img[ecore, dp, ecol] = pairrow_of_node[src]
    w_img[ecore, dp, 2 * ecol + half_of_node[src]] = dinv[src]

    # wrapped int16 idx per chunk: slot m = (col-c0)*128 + p at [m%16, m//16], x8
    gidxw = np.empty((n_cores, P, WSUM), np.int16)
    for (bi, c0, c1, w0) in chunks:
        o = bat_base[bi] + c0
        cw = c1 - c0
        flat = idx_img[:, :, o:o + cw]                    # [c, p, col]
        flat = flat.transpose(0, 2, 1).reshape(n_cores, P * cw)  # m = col*128+p
        wrapped = flat.reshape(n_cores, 8 * cw, 16).transpose(0, 2, 1)
        gidxw[:, :, w0:w0 + 8 * cw] = np.tile(wrapped, (1, 8, 1)).astype(np.int16)

    # dinv per (p, g) position
    dinv_arr = np.zeros((n_cores, P, G), np.float32)
    xt = np.zeros((n_cores, F_IN, NPC_PAD), np.float32)
    for c in range(n_cores):
        loc = perms[c]
        valid = loc >= 0
        v = np.zeros(NPC_PAD, np.float32)
        v[valid] = dinv[c * NPC + loc[valid]]
        dinv_arr[c] = v.reshape(G, P).T
        xt[c][:, valid] = x[c * NPC + loc[valid]].T

    b1_in = b1.reshape(HID // P, P).T.copy()  # [128, HID//128]
    b2_in = b2[:, None].copy()                # [CLS, 1]

    cfg = dict(
        N=N, F_IN=F_IN, HID=HID, CLS=CLS, K=K, NPC=NPC, NPC_PAD=NPC_PAD,
        G=G, V2=V2, PAIRS_PC=PAIRS_PC, SUMC=SUMC, WSUM=WSUM,
        batches=tuple(batches), bat_base=tuple(int(v) for v in bat_base),
        chunks=tuple(chunks),
        gamma=tuple(float(v) for v in gamma), n_cores=n_cores,
    )
    in_maps = []
    for c in range(n_cores):
        in_maps.append({
            "xt": np.ascontiguousarray(xt[c]).astype(BF16),
            "w1": W1.astype(BF16), "b1": b1_in, "w2": W2.astype(BF16),
            "b2": b2_in,
            "dinv": np.ascontiguousarray(dinv_arr[c]).astype(BF16),
            "dinv2": np.ascontiguousarray(dinv_arr[c] ** 2).astype(BF16),
            "gidx": np.ascontiguousarray(gidxw[c]),
            "wslot": np.ascontiguousarray(w_img[c]).astype(BF16),
        })
    return cfg, in_maps, perms


def postprocess(cfg, perms, outs):
    N, G, NPC, NPC_PAD = cfg["N"], cfg["G"], cfg["NPC"], cfg["NPC_PAD"]
    res = np.empty((N, CLS), np.float32)
    for c in range(cfg["n_cores"]):
        arr = np.asarray(outs[c]).reshape(P, G, CLS)
        zpos = arr.transpose(1, 0, 2).reshape(NPC_PAD, CLS)  # pos = g*128+p
        loc = perms[c]
        valid = loc >= 0
        res[c * NPC + loc[valid]] = zpos[valid]
    return res


# ---------------------------------------------------------------------------
# Device graph
# ---------------------------------------------------------------------------

def build_graph(cfg):
    import concourse.bacc as bacc
    import concourse.mybir as mybir
    import concourse.tile as tile
    from concourse.masks import make_identity

    f32 = mybir.dt.float32
    bf16 = mybir.dt.bfloat16
    i16 = mybir.dt.int16
    Alu = mybir.AluOpType
    Act = mybir.ActivationFunctionType

    F_IN, HID, K = cfg["F_IN"], cfg["HID"], cfg["K"]
    NPC_PAD, G, V2 = cfg["NPC_PAD"], cfg["G"], cfg["V2"]
    SUMC, WSUM = cfg["SUMC"], cfg["WSUM"]
    batches, bat_base, chunks = cfg["batches"], cfg["bat_base"], cfg["chunks"]
    chunks_of = {}
    for (bi, c0, c1, w0) in chunks:
        chunks_of.setdefault(bi, []).append((c0, c1, w0))
    gamma = cfg["gamma"]
    n_cores = cfg["n_cores"]
    KF = F_IN // P
    KH = HID // P

    nc = bacc.Bacc("TRN2", target_bir_lowering=False, debug=False,
                   enable_asserts=False, num_devices=n_cores,
                   num_swdge_queues=4)

    xt_d = nc.dram_tensor("xt", [F_IN, NPC_PAD], bf16, kind="ExternalInput")
    w1_d = nc.dram_tensor("w1", [F_IN, HID], bf16, kind="ExternalInput")
    b1_d = nc.dram_tensor("b1", [P, KH], f32, kind="ExternalInput")
    w2_d = nc.dram_tensor("w2", [HID, CLS], bf16, kind="ExternalInput")
    b2_d = nc.dram_tensor("b2", [CLS, 1], f32, kind="ExternalInput")
    dinv_d = nc.dram_tensor("dinv", [P, G], bf16, kind="ExternalInput")
    dinv2_d = nc.dram_tensor("dinv2", [P, G], bf16, kind="ExternalInput")
    gidx_d = nc.dram_tensor("gidx", [P, WSUM], i16, kind="ExternalInput")
    wslot_d = nc.dram_tensor("wslot", [P, 2 * SUMC], bf16, kind="ExternalInput")
    out_d = nc.dram_tensor("out", [P, G * CLS], f32, kind="ExternalOutput")

    bounce = [nc.dram_tensor(f"hsb{i}", [NPC_PAD * CLS], bf16) for i in range(2)]
    tables = [nc.dram_tensor(f"table{i}", [V2, 2 * CLS], bf16) for i in range(2)]
    groups = [list(range(n_cores))]

    with tile.TileContext(nc, num_cores=n_cores) as tc:
        with (
            tc.tile_pool(name="persist", bufs=1) as pp,
            tc.tile_pool(name="ps", bufs=2, space="PSUM") as psp,
        ):
            # ---- persistent tiles ----
            w1_sb = pp.tile([P, KF, HID], bf16)
            nc.sync.dma_start(w1_sb[:], w1_d.ap().rearrange("(k p) h -> p k h", p=P))
            w2_sb = pp.tile([P, KH, CLS], bf16)
            nc.sync.dma_start(w2_sb[:], w2_d.ap().rearrange("(k p) h -> p k h", p=P))
            b1_sb = pp.tile([P, KH], f32)
            nc.sync.dma_start(b1_sb[:], b1_d[:, :])
            b2_sb = pp.tile([CLS, 1], f32)
            nc.sync.dma_start(b2_sb[:], b2_d[:, :])
            dinv_sb = pp.tile([P, G], bf16)
            nc.sync.dma_start(dinv_sb[:], dinv_d[:, :])
            dinv2_sb = pp.tile([P, G], bf16)
            nc.sync.dma_start(dinv2_sb[:], dinv2_d[:, :])
            idx_sb = pp.tile([P, WSUM], i16)
            nc.sync.dma_start(idx_sb[:], gidx_d[:, :])
            wslot_sb = pp.tile([P, 2 * SUMC], bf16)
            nc.sync.dma_start(wslot_sb[:], wslot_d[:, :])
            ident = pp.tile([P, P], f32)
            make_identity(nc, ident[:])
            hn_sb = pp.tile([P, G, CLS], bf16)
            z_sb = pp.tile([P, G, CLS], f32)

            # ---- MLP ----
            g0f = float(gamma[0])
            with tc.tile_pool(name="mlp", bufs=2) as mp:
                col = 0
                while col < NPC_PAD:
                    F = min(512, NPC_PAD - col)
                    xk = mp.tile([P, KF, F], bf16, tag="xk")
                    nc.sync.dma_start(
                        xk[:],
                        xt_d.ap().rearrange("(k p) n -> p k n", p=P)[:, :, col:col + F])
                    h1 = []
                    for c2 in range(KH):
                        ps1 = psp.tile([P, F], f32, tag=f"ps1_{c2}")
                        for k in range(KF):
                            nc.tensor.matmul(ps1[:], lhsT=w1_sb[:, k, c2 * P:(c2 + 1) * P],
                                             rhs=xk[:, k, :], start=(k == 0),
                                             stop=(k == KF - 1))
                        h1c = mp.tile([P, F], bf16, tag=f"h1_{c2}")
                        nc.scalar.activation(h1c[:], ps1[:], Act.Relu,
                                             bias=b1_sb[:, c2:c2 + 1], scale=1.0)
                        h1.append(h1c)
                    ps2 = psp.tile([CLS, F], f32, tag="ps2")
                    for c2 in range(KH):
                        nc.tensor.matmul(ps2[:], lhsT=w2_sb[:, c2, :], rhs=h1[c2][:],
                                         start=(c2 == 0), stop=(c2 == KH - 1))
                    h2t = mp.tile([CLS, F], f32, tag="h2t")
                    nc.scalar.activation(h2t[:], ps2[:], Act.Identity, bias=b2_sb[:, 0:1])
                    for gg in range(F // P):
                        g = (col + gg * P) // P
                        pst = psp.tile([P, CLS], f32, tag="pst")
                        nc.tensor.transpose(pst[:], in_=h2t[:, gg * P:(gg + 1) * P],
                                            identity=ident[:CLS, :CLS])
                        nc.vector.tensor_scalar_mul(z_sb[:, g, :], pst[:], g0f)
                        nc.vector.tensor_copy(hn_sb[:, g, :], pst[:])
                    col += F

            nc.sync.dma_start(bounce[0].ap().rearrange("(p x) -> p x", p=P), hn_sb[:])
            nc.gpsimd.collective_compute(
                "AllGather", Alu.bypass, replica_groups=groups,
                ins=[bounce[0].ap().opt()], outs=[tables[0].ap().opt()])

            # ---- hops ----
            MCB = max(nt * lb for (_, nt, lb) in batches)
            NTM = max(nt for (_, nt, lb) in batches)
            with (
                tc.tile_pool(name="gat", bufs=2) as gp,
                tc.tile_pool(name="upd", bufs=2) as up,
                nc.allow_low_precision(reason="bf16 tree reduce; tol 2e-2"),
            ):
                qn = 0
                for k in range(1, K + 1):
                    tbl = tables[(k - 1) % 2]
                    gk = float(gamma[k])
                    for bi, (g0, nt, lb) in enumerate(batches):
                        cb = nt * lb
                        m = gp.tile([P, MCB, 2 * CLS], bf16, tag="m")
                        for (c0, c1, w0) in chunks_of[bi]:
                            cw = c1 - c0
                            nc.gpsimd.dma_gather(
                                m[:, c0:c1, :], tbl[0:V2, :],
                                idx_sb[:, w0:w0 + 8 * cw],
                                num_idxs=P * cw, num_idxs_reg=P * cw,
                                elem_size=2 * CLS, single_packet=False,
                                queue_num=qn % 4)
                            qn += 1
                        # virtual slots: [128, cb, 128] -> [128, 2cb, 64];
                        # virtual col = 2*slotcol + half
                        mv = m[:, 0:cb, :].rearrange("p c (t f) -> p (c t) f",
                                                     t=2, f=CLS)
                        o2 = 2 * bat_base[bi]
                        wv = wslot_sb[:, o2:o2 + 2 * cb].rearrange(
                            "p (v o) -> p v o", o=1
                        ).to_broadcast([P, 2 * cb, CLS])
                        nc.vector.tensor_tensor(mv, mv, wv, op=Alu.mult)
                        # per-tile view [128, nt, 2lb, 64]; in-place bf16 tree
                        v4 = mv.rearrange("p (n v) f -> p n v f", n=nt)
                        L = 2 * lb
                        while L > 1:
                            p2 = 1 << (L.bit_length() - 1)
                            if p2 == L:
                                p2 = L // 2
                            rem = L - p2
                            nc.vector.tensor_tensor(
                                v4[:, :, 0:rem, :], v4[:, :, 0:rem, :],
                                v4[:, :, p2:L, :], op=Alu.add)
                            L = p2
                        acc = v4[:, :, 0, :]                      # [128, nt, 64] bf16
                        dv = dinv_sb[:, g0:g0 + nt].rearrange(
                            "p (n o) -> p n o", o=1).to_broadcast([P, nt, CLS])
                        dv2 = dinv2_sb[:, g0:g0 + nt].rearrange(
                            "p (n o) -> p n o", o=1).to_broadcast([P, nt, CLS])
                        t1 = up.tile([P, NTM, CLS], f32, tag="t1")
                        t2 = up.tile([P, NTM, CLS], f32, tag="t2")
                        nc.vector.tensor_tensor(t1[:, 0:nt, :], acc, dv, op=Alu.mult)
                        nc.vector.tensor_tensor(t2[:, 0:nt, :],
                                                hn_sb[:, g0:g0 + nt, :], dv2,
                                                op=Alu.mult)
                        nc.vector.tensor_tensor(t1[:, 0:nt, :], t1[:, 0:nt, :],
                                                t2[:, 0:nt, :], op=Alu.add)
                        nc.vector.tensor_copy(hn_sb[:, g0:g0 + nt, :], t1[:, 0:nt, :])
                        if gk != 0.0:
                            nc.vector.scalar_tensor_tensor(
                                z_sb[:, g0:g0 + nt, :], in0=t1[:, 0:nt, :], scalar=gk,
                                in1=z_sb[:, g0:g0 + nt, :],
                                op0=Alu.mult, op1=Alu.add)
                    if k < K:
                        bb = bounce[k % 2]
                        nc.sync.dma_start(bb.ap().rearrange("(p x) -> p x", p=P),
                                          hn_sb[:])
                        nc.gpsimd.collective_compute(
                            "AllGather", Alu.bypass, replica_groups=groups,
                            ins=[bb.ap().opt()], outs=[tables[k % 2].ap().opt()])

            # ---- log_softmax ----
            rmax = pp.tile([P, G], f32)
            nc.vector.tensor_reduce(rmax[:], z_sb[:], axis=mybir.AxisListType.X,
                                    op=Alu.max)
            rmb = rmax[:].rearrange("p (g o) -> p g o", o=1).to_broadcast([P, G, CLS])
            nc.vector.tensor_tensor(z_sb[:], z_sb[:], rmb, op=Alu.subtract)
            e_sb = pp.tile([P, G, CLS], f32)
            nc.scalar.activation(e_sb[:], z_sb[:], Act.Exp)
            rsum = pp.tile([P, G], f32)
            nc.vector.tensor_reduce(rsum[:], e_sb[:], axis=mybir.AxisListType.X,
                                    op=Alu.add)
            lsum = pp.tile([P, G], f32)
            nc.scalar.activation(lsum[:], rsum[:], Act.Ln)
            lsb = lsum[:].rearrange("p (g o) -> p g o", o=1).to_broadcast([P, G, CLS])
            nc.vector.tensor_tensor(z_sb[:], z_sb[:], lsb, op=Alu.subtract)
            nc.sync.dma_start(out_d[:, :], z_sb[:])

    nc.finalize()
    return nc


# ---------------------------------------------------------------------------
# Entry point
# ---------------------------------------------------------------------------

def run(cfg, in_maps, perms, **spmd_kwargs):
    import concourse.bass_utils as bass_utils
    nc = build_graph(cfg)
    res = bass_utils.run_bass_kernel_spmd(
        nc, in_maps, core_ids=list(range(cfg["n_cores"])), **spmd_kwargs)
    return postprocess(cfg, perms, [r["out"] for r in res.results]), res


def kernel(x, edge_index, W1, b1, W2, b2, gamma):
    cfg, in_maps, perms = preprocess(x, edge_index, W1, b1, W2, b2, gamma)
    out, _ = run(cfg, in_maps, perms)
    return out


# revision 14
# speedup vs baseline: 1.1229x; 1.1229x over previous
"""Trainium2 Bass kernel for EvenNet GNN message passing, SPMD across 8 NeuronCores.

v2 strategy (vs baseline): the baseline was bottlenecked by SWDGE descriptor
generation on the GpSimd/Pool engine (measured 4.7us/call + 1.5ns/idx,
98 calls/hop, 253k idx/hop -> 8.5ms of 12.3ms).  This version:

  - bf16 pair-row feature table: row p = [h(2p) | h(2p+1)] (128 bf16 =
    256B).  Pair count 25600 < 32768 so a single int16 index window covers
    the table (the baseline needed a two-window A/B split with balanced
    padding: 253k -> ~210k idx).
  - Per-slot bf16 weights w = dinv[src] * (parity mask of the wanted
    half); the unwanted half and all pad slots get weight 0.  The slot
    reduce becomes: multiply the gathered [128, C, 128] tile (viewed as
    [128, NT, 2L, 64]) by broadcast weights, then an in-place bf16
    halving-tree over the slot axis.  Few large DVE instructions instead
    of many small ones.
  - Gather calls batched over many tiles (~8 calls/hop instead of 98).
  - bf16 MLP (4x tensor throughput), bf16 AllGather (half the bytes).
  - log_softmax with broadcast ops (6 instrs instead of ~100).

Layout: nodes degree-sorted per core; position pos in [0, 6400);
partition p = pos % 128, tile g = pos // 128 (G = 50 tiles).  Table pair
row (global) = core*3200 + p*25 + (g>>1); half = g & 1.  hn SBUF tile
[128, 50, 64] bf16 dumps per-partition-contiguously = exactly the pair
table shard layout.
"""

import numpy as np

try:
    import ml_dtypes
    BF16 = ml_dtypes.bfloat16
except ImportError:  # pragma: no cover
    BF16 = None

N_CORES = 8
P = 128
CLS = 64
COLS_PER_BATCH = 112  # max NT*L slot columns per reduce batch (SBUF budget)
NBUF = 3              # manual rotation depth for gather buffers


# ---------------------------------------------------------------------------
# Host preprocessing
# ---------------------------------------------------------------------------

def preprocess(x, edge_index, W1, b1, W2, b2, gamma, n_cores=N_CORES):
    x = np.ascontiguousarray(np.asarray(x, np.float32))
    edge_index = np.asarray(edge_index)
    W1 = np.asarray(W1, np.float32)
    b1 = np.asarray(b1, np.float32)
    W2 = np.asarray(W2, np.float32)
    b2 = np.asarray(b2, np.float32)
    gamma = np.asarray(gamma, np.float32)

    N, F_IN = x.shape
    HID = W1.shape[1]
    K = gamma.shape[0] - 1
    assert N % n_cores == 0
    NPC = N // n_cores
    G = -(-(NPC + 1) // (2 * P)) * 2  # tiles, rounded to even (pair rows)
    NPC_PAD = G * P
    PAIRS_PC = NPC_PAD // 2          # pair rows per core
    V2 = n_cores * PAIRS_PC          # total pair rows
    assert V2 < 32768, V2

    src = edge_index[0].astype(np.int64)
    dst = edge_index[1].astype(np.int64)
    E = src.shape[0]

    deg = (np.bincount(dst, minlength=N) + 1.0).astype(np.float32)  # + self
    dinv = (1.0 / np.sqrt(deg)).astype(np.float32)

    # degree-sorted positions per core
    perms = []
    pos_of_node = np.empty(N, np.int64)
    for c in range(n_cores):
        lo = c * NPC
        order = np.argsort(-deg[lo:lo + NPC], kind="stable")
        perm = np.full(NPC_PAD, -1, np.int64)
        perm[:NPC] = order
        perms.append(perm)
        invp = np.empty(NPC, np.int64)
        invp[order] = np.arange(NPC)
        pos_of_node[lo:lo + NPC] = invp

    core_of_node = np.arange(N) // NPC
    p_of_node = pos_of_node % P
    g_of_node = pos_of_node // P
    # global pair row + half for every node (as gather source)
    pairrow_of_node = (core_of_node * PAIRS_PC + p_of_node * (G // 2)
                       + (g_of_node >> 1)).astype(np.int32)
    half_of_node = (g_of_node & 1).astype(np.int64)

    # per-edge dst coordinates
    ecore = core_of_node[dst]
    dp = p_of_node[dst]
    dg = g_of_node[dst]

    # slot index within dst, ordered by source pair row (HBM locality)
    okey = (dst << 16) | pairrow_of_node[src].astype(np.int64)
    order_e = np.argsort(okey, kind="stable")
    sd = dst[order_e]
    first = np.r_[True, sd[1:] != sd[:-1]]
    idx_first = np.where(first)[0]
    run_starts = np.repeat(idx_first, np.diff(np.r_[idx_first, len(sd)]))
    l_sorted = np.arange(len(sd)) - run_starts
    l_arr = np.empty(E, np.int64)
    l_arr[order_e] = l_sorted

    # per-tile slot count L[g]: max over cores and partitions
    degi = deg.astype(np.int64) - 1  # in-edge count (no self)
    Lg = np.zeros(G, np.int64)
    np.maximum.at(Lg, g_of_node, degi)
    Lg = np.maximum(Lg, 1)

    # greedy batches of consecutive tiles (degree-desc => L non-increasing)
    batches = []  # (g0, NT, Lb)
    g0 = 0
    while g0 < G:
        Lb = Lg[g0]
        nt = 1
        while g0 + nt < G and (nt + 1) * Lb <= COLS_PER_BATCH:
            nt += 1
        batches.append((g0, nt, int(Lb)))
        g0 += nt

    # column offsets
    col_off = np.zeros(G, np.int64)      # slot-column base of tile g
    bat_base = []                        # idx-column base of each batch
    acc = 0
    for (g0, nt, lb) in batches:
        bat_base.append(acc)
        for i in range(nt):
            col_off[g0 + i] = acc + i * lb
        acc += nt * lb
    SUMC = acc                           # total slot columns

    # gather calls are chunks of <= CHUNK_COLS columns (SWDGE ring capacity)
    CHUNK_COLS = 56
    chunks = []   # (batch_idx, c0, c1, wrapped_col_base)
    wacc = 0
    for bi, (g0, nt, lb) in enumerate(batches):
        cb = nt * lb
        c0 = 0
        while c0 < cb:
            c1 = min(c0 + CHUNK_COLS, cb)
            chunks.append((bi, c0, c1, wacc))
            wacc += 8 * (c1 - c0)
            c0 = c1
    WSUM = wacc

    # idx and weight images
    idx_img = np.zeros((n_cores, P, SUMC), np.int32)
    w_img = np.zeros((n_cores, P, 2 * SUMC), np.float32)
    ecol = col_off[dg] + l_arr           # slot column of each edge
    idx_img[ecore, dp, ecol] = pairrow_of_node[src]
    w_img[ecore, dp, 2 * ecol + half_of_node[src]] = dinv[src]

    # wrapped int16 idx per chunk: slot m = (col-c0)*128 + p at [m%16, m//16], x8
    gidxw = np.empty((n_cores, P, WSUM), np.int16)
    for (bi, c0, c1, w0) in chunks:
        o = bat_base[bi] + c0
        cw = c1 - c0
        flat = idx_img[:, :, o:o + cw]                    # [c, p, col]
        flat = flat.transpose(0, 2, 1).reshape(n_cores, P * cw)  # m = col*128+p
        wrapped = flat.reshape(n_cores, 8 * cw, 16).transpose(0, 2, 1)
        gidxw[:, :, w0:w0 + 8 * cw] = np.tile(wrapped, (1, 8, 1)).astype(np.int16)

    # dinv per (p, g) position
    dinv_arr = np.zeros((n_cores, P, G), np.float32)
    xt = np.zeros((n_cores, F_IN, NPC_PAD), np.float32)
    for c in range(n_cores):
        loc = perms[c]
        valid = loc >= 0
        v = np.zeros(NPC_PAD, np.float32)
        v[valid] = dinv[c * NPC + loc[valid]]
        dinv_arr[c] = v.reshape(G, P).T
        xt[c][:, valid] = x[c * NPC + loc[valid]].T

    b1_in = b1.reshape(HID // P, P).T.copy()  # [128, HID//128]
    b2_in = b2[:, None].copy()                # [CLS, 1]

    cfg = dict(
        N=N, F_IN=F_IN, HID=HID, CLS=CLS, K=K, NPC=NPC, NPC_PAD=NPC_PAD,
        G=G, V2=V2, PAIRS_PC=PAIRS_PC, SUMC=SUMC, WSUM=WSUM,
        batches=tuple(batches), bat_base=tuple(int(v) for v in bat_base),
        chunks=tuple(chunks),
        gamma=tuple(float(v) for v in gamma), n_cores=n_cores,
    )
    in_maps = []
    for c in range(n_cores):
        in_maps.append({
            "xt": np.ascontiguousarray(xt[c]).astype(BF16),
            "w1": W1.astype(BF16), "b1": b1_in, "w2": W2.astype(BF16),
            "b2": b2_in,
            "dinv": np.ascontiguousarray(dinv_arr[c]).astype(BF16),
            "dinv2": np.ascontiguousarray(dinv_arr[c] ** 2).astype(BF16),
            "gidx": np.ascontiguousarray(gidxw[c]),
            "wslot": np.ascontiguousarray(w_img[c]).astype(BF16),
        })
    return cfg, in_maps, perms


def postprocess(cfg, perms, outs):
    N, G, NPC, NPC_PAD = cfg["N"], cfg["G"], cfg["NPC"], cfg["NPC_PAD"]
    res = np.empty((N, CLS), np.float32)
    for c in range(cfg["n_cores"]):
        arr = np.asarray(outs[c]).reshape(P, G, CLS)
        zpos = arr.transpose(1, 0, 2).reshape(NPC_PAD, CLS)  # pos = g*128+p
        loc = perms[c]
        valid = loc >= 0
        res[c * NPC + loc[valid]] = zpos[valid]
    return res


# ---------------------------------------------------------------------------
# Device graph
# ---------------------------------------------------------------------------

def build_graph(cfg):
    import concourse.bacc as bacc
    import concourse.mybir as mybir
    import concourse.tile as tile
    from concourse.masks import make_identity

    f32 = mybir.dt.float32
    bf16 = mybir.dt.bfloat16
    i16 = mybir.dt.int16
    Alu = mybir.AluOpType
    Act = mybir.ActivationFunctionType

    F_IN, HID, K = cfg["F_IN"], cfg["HID"], cfg["K"]
    NPC_PAD, G, V2 = cfg["NPC_PAD"], cfg["G"], cfg["V2"]
    SUMC, WSUM = cfg["SUMC"], cfg["WSUM"]
    batches, bat_base, chunks = cfg["batches"], cfg["bat_base"], cfg["chunks"]
    chunks_of = {}
    for (bi, c0, c1, w0) in chunks:
        chunks_of.setdefault(bi, []).append((c0, c1, w0))
    gamma = cfg["gamma"]
    n_cores = cfg["n_cores"]
    KF = F_IN // P
    KH = HID // P

    nc = bacc.Bacc("TRN2", target_bir_lowering=False, debug=False,
                   enable_asserts=False, num_devices=n_cores,
                   num_swdge_queues=4)

    xt_d = nc.dram_tensor("xt", [F_IN, NPC_PAD], bf16, kind="ExternalInput")
    w1_d = nc.dram_tensor("w1", [F_IN, HID], bf16, kind="ExternalInput")
    b1_d = nc.dram_tensor("b1", [P, KH], f32, kind="ExternalInput")
    w2_d = nc.dram_tensor("w2", [HID, CLS], bf16, kind="ExternalInput")
    b2_d = nc.dram_tensor("b2", [CLS, 1], f32, kind="ExternalInput")
    dinv_d = nc.dram_tensor("dinv", [P, G], bf16, kind="ExternalInput")
    dinv2_d = nc.dram_tensor("dinv2", [P, G], bf16, kind="ExternalInput")
    gidx_d = nc.dram_tensor("gidx", [P, WSUM], i16, kind="ExternalInput")
    wslot_d = nc.dram_tensor("wslot", [P, 2 * SUMC], bf16, kind="ExternalInput")
    out_d = nc.dram_tensor("out", [P, G * CLS], f32, kind="ExternalOutput")

    bounce = [nc.dram_tensor(f"hsb{i}", [NPC_PAD * CLS], bf16) for i in range(2)]
    tables = [nc.dram_tensor(f"table{i}", [V2, 2 * CLS], bf16) for i in range(2)]
    groups = [list(range(n_cores))]

    with tile.TileContext(nc, num_cores=n_cores) as tc:
        with (
            tc.tile_pool(name="persist", bufs=1) as pp,
            tc.tile_pool(name="ps", bufs=2, space="PSUM") as psp,
        ):
            # ---- persistent tiles ----
            w1_sb = pp.tile([P, KF, HID], bf16)
            nc.sync.dma_start(w1_sb[:], w1_d.ap().rearrange("(k p) h -> p k h", p=P))
            w2_sb = pp.tile([P, KH, CLS], bf16)
            nc.sync.dma_start(w2_sb[:], w2_d.ap().rearrange("(k p) h -> p k h", p=P))
            b1_sb = pp.tile([P, KH], f32)
            nc.sync.dma_start(b1_sb[:], b1_d[:, :])
            b2_sb = pp.tile([CLS, 1], f32)
            nc.sync.dma_start(b2_sb[:], b2_d[:, :])
            dinv_sb = pp.tile([P, G], bf16)
            nc.sync.dma_start(dinv_sb[:], dinv_d[:, :])
            dinv2_sb = pp.tile([P, G], bf16)
            nc.sync.dma_start(dinv2_sb[:], dinv2_d[:, :])
            idx_sb = pp.tile([P, WSUM], i16)
            nc.sync.dma_start(idx_sb[:], gidx_d[:, :])
            wslot_sb = pp.tile([P, 2 * SUMC], bf16)
            nc.sync.dma_start(wslot_sb[:], wslot_d[:, :])
            ident = pp.tile([P, P], f32)
            make_identity(nc, ident[:])
            hn_sb = pp.tile([P, G, CLS], bf16)
            z_sb = pp.tile([P, G, CLS], f32)

            # ---- MLP ----
            g0f = float(gamma[0])
            with tc.tile_pool(name="mlp", bufs=2) as mp:
                col = 0
                while col < NPC_PAD:
                    F = min(512, NPC_PAD - col)
                    xk = mp.tile([P, KF, F], bf16, tag="xk")
                    nc.sync.dma_start(
                        xk[:],
                        xt_d.ap().rearrange("(k p) n -> p k n", p=P)[:, :, col:col + F])
                    h1 = []
                    for c2 in range(KH):
                        ps1 = psp.tile([P, F], f32, tag=f"ps1_{c2}")
                        for k in range(KF):
                            nc.tensor.matmul(ps1[:], lhsT=w1_sb[:, k, c2 * P:(c2 + 1) * P],
                                             rhs=xk[:, k, :], start=(k == 0),
                                             stop=(k == KF - 1))
                        h1c = mp.tile([P, F], bf16, tag=f"h1_{c2}")
                        nc.scalar.activation(h1c[:], ps1[:], Act.Relu,
                                             bias=b1_sb[:, c2:c2 + 1], scale=1.0)
                        h1.append(h1c)
                    ps2 = psp.tile([CLS, F], f32, tag="ps2")
                    for c2 in range(KH):
                        nc.tensor.matmul(ps2[:], lhsT=w2_sb[:, c2, :], rhs=h1[c2][:],
                                         start=(c2 == 0), stop=(c2 == KH - 1))
                    h2t = mp.tile([CLS, F], f32, tag="h2t")
                    nc.scalar.activation(h2t[:], ps2[:], Act.Identity, bias=b2_sb[:, 0:1])
                    for gg in range(F // P):
                        g = (col + gg * P) // P
                        pst = psp.tile([P, CLS], f32, tag="pst")
                        nc.tensor.transpose(pst[:], in_=h2t[:, gg * P:(gg + 1) * P],
                                            identity=ident[:CLS, :CLS])
                        nc.vector.tensor_scalar_mul(z_sb[:, g, :], pst[:], g0f)
                        nc.vector.tensor_copy(hn_sb[:, g, :], pst[:])
                    col += F

            nc.sync.dma_start(bounce[0].ap().rearrange("(p x) -> p x", p=P), hn_sb[:])
            nc.gpsimd.collective_compute(
                "AllGather", Alu.bypass, replica_groups=groups,
                ins=[bounce[0].ap().opt()], outs=[tables[0].ap().opt()])

            # ---- hops ----
            MCB = max(nt * lb for (_, nt, lb) in batches)
            NTM = max(nt for (_, nt, lb) in batches)
            with (
                tc.tile_pool(name="gat", bufs=1) as gp,
                tc.tile_pool(name="upd", bufs=2) as up,
                nc.allow_low_precision(reason="bf16 tree reduce; tol 2e-2"),
            ):
                # manual rotation over NBUF gather buffers
                mbufs = [gp.tile([P, MCB, 2 * CLS], bf16, tag=f"m{i}",
                                 name=f"mbuf{i}")
                         for i in range(NBUF)]
                qn = 0
                for k in range(1, K + 1):
                    tbl = tables[(k - 1) % 2]
                    gk = float(gamma[k])
                    for bi, (g0, nt, lb) in enumerate(batches):
                        cb = nt * lb
                        m = mbufs[(k * len(batches) + bi) % NBUF]
                        o2 = 2 * bat_base[bi]
                        for (c0, c1, w0) in chunks_of[bi]:
                            cw = c1 - c0
                            nc.gpsimd.dma_gather(
                                m[:, c0:c1, :], tbl[0:V2, :],
                                idx_sb[:, w0:w0 + 8 * cw],
                                num_idxs=P * cw, num_idxs_reg=P * cw,
                                elem_size=2 * CLS, single_packet=False,
                                queue_num=qn % 4)
                            qn += 1
                            # weight-multiply this chunk as soon as it lands:
                            # [128, cw, 128] -> [128, 2cw, 64] virtual slots
                            mc = m[:, c0:c1, :].rearrange(
                                "p c (t f) -> p (c t) f", t=2, f=CLS)
                            wc = wslot_sb[:, o2 + 2 * c0:o2 + 2 * c1].rearrange(
                                "p (v o) -> p v o", o=1
                            ).to_broadcast([P, 2 * cw, CLS])
                            nc.vector.tensor_tensor(mc, mc, wc, op=Alu.mult)
                        # per-tile view [128, nt, 2lb, 64]; in-place bf16 tree
                        v4 = m[:, 0:cb, :].rearrange(
                            "p (n l) (t f) -> p n (l t) f", n=nt, t=2, f=CLS)
                        L = 2 * lb
                        while L > 1:
                            p2 = 1 << (L.bit_length() - 1)
                            if p2 == L:
                                p2 = L // 2
                            rem = L - p2
                            nc.vector.tensor_tensor(
                                v4[:, :, 0:rem, :], v4[:, :, 0:rem, :],
                                v4[:, :, p2:L, :], op=Alu.add)
                            L = p2
                        acc = v4[:, :, 0, :]                      # [128, nt, 64] bf16
                        dv = dinv_sb[:, g0:g0 + nt].rearrange(
                            "p (n o) -> p n o", o=1).to_broadcast([P, nt, CLS])
                        dv2 = dinv2_sb[:, g0:g0 + nt].rearrange(
                            "p (n o) -> p n o", o=1).to_broadcast([P, nt, CLS])
                        t1 = up.tile([P, NTM, CLS], f32, tag="t1")
                        t2 = up.tile([P, NTM, CLS], f32, tag="t2")
                        nc.vector.tensor_tensor(t1[:, 0:nt, :], acc, dv, op=Alu.mult)
                        nc.vector.tensor_tensor(t2[:, 0:nt, :],
                                                hn_sb[:, g0:g0 + nt, :], dv2,
                                                op=Alu.mult)
                        nc.vector.tensor_tensor(t1[:, 0:nt, :], t1[:, 0:nt, :],
                                                t2[:, 0:nt, :], op=Alu.add)
                        nc.vector.tensor_copy(hn_sb[:, g0:g0 + nt, :], t1[:, 0:nt, :])
                        if gk != 0.0:
                            nc.vector.scalar_tensor_tensor(
                                z_sb[:, g0:g0 + nt, :], in0=t1[:, 0:nt, :], scalar=gk,
                                in1=z_sb[:, g0:g0 + nt, :],
                                op0=Alu.mult, op1=Alu.add)
                    if k < K:
                        bb = bounce[k % 2]
                        nc.sync.dma_start(bb.ap().rearrange("(p x) -> p x", p=P),
                                          hn_sb[:])
                        nc.gpsimd.collective_compute(
                            "AllGather", Alu.bypass, replica_groups=groups,
                            ins=[bb.ap().opt()], outs=[tables[k % 2].ap().opt()])

            # ---- log_softmax ----
            rmax = pp.tile([P, G], f32)
            nc.vector.tensor_reduce(rmax[:], z_sb[:], axis=mybir.AxisListType.X,
                                    op=Alu.max)
            rmb = rmax[:].rearrange("p (g o) -> p g o", o=1).to_broadcast([P, G, CLS])
            nc.vector.tensor_tensor(z_sb[:], z_sb[:], rmb, op=Alu.subtract)
            e_sb = pp.tile([P, G, CLS], f32)
            nc.scalar.activation(e_sb[:], z_sb[:], Act.Exp)
            rsum = pp.tile([P, G], f32)
            nc.vector.tensor_reduce(rsum[:], e_sb[:], axis=mybir.AxisListType.X,
                                    op=Alu.add)
            lsum = pp.tile([P, G], f32)
            nc.scalar.activation(lsum[:], rsum[:], Act.Ln)
            lsb = lsum[:].rearrange("p (g o) -> p g o", o=1).to_broadcast([P, G, CLS])
            nc.vector.tensor_tensor(z_sb[:], z_sb[:], lsb, op=Alu.subtract)
            nc.sync.dma_start(out_d[:, :], z_sb[:])

    nc.finalize()
    return nc


# ---------------------------------------------------------------------------
# Entry point
# ---------------------------------------------------------------------------

def run(cfg, in_maps, perms, **spmd_kwargs):
    import concourse.bass_utils as bass_utils
    nc = build_graph(cfg)
    res = bass_utils.run_bass_kernel_spmd(
        nc, in_maps, core_ids=list(range(cfg["n_cores"])), **spmd_kwargs)
    return postprocess(cfg, perms, [r["out"] for r in res.results]), res


def kernel(x, edge_index, W1, b1, W2, b2, gamma):
    cfg, in_maps, perms = preprocess(x, edge_index, W1, b1, W2, b2, gamma)
    out, _ = run(cfg, in_maps, perms)
    return out
